# revision 5
# baseline (speedup 1.0000x reference)
"""MinGRU cell on 8 Trainium2 NeuronCores.

Math: per (batch b, hidden channel j), the reference computes (in log space)
the linear recurrence

    h_t = c_t * h_{t-1} + v_t,      h_0 = g(h0)
    c_t = 1 - sigmoid(kz_t) = sigmoid(-kz_t)
    v_t = z_t * g(kh_t),  z_t = 1 - c_t
    kz = x @ Wz^T + bz,  kh = x @ Wh^T + bh
    g(u) = max(sigmoid(u), u + 0.5)   (exact identity for the reference's g)

All quantities are positive and O(1), so the linear-space recurrence in fp32
with fp16 intermediates is accurate to ~2e-3 (verified vs the log-space
reference; tolerance is 2e-2).

Matmul precision/speed: fp8(e4m3) with MatmulPerfMode.DoubleRow. One DR
instruction computes sum_i W[:,i].T @ X[:,i] over the two "slots" at 0.5
cycles per output row (2x the fp32r rate per instruction).
  - "pair pass": slots = (x_hi, x_lo) against duplicated W8 -> (x_hi+x_lo)@W8
    with x quantization error ~2^-9. 8 instructions per K=1024.
  - "lo pass" (scheme D): slots = two k-tiles of x_hi against W_lo8 where
    W_lo8 = q8(32W - W8): 4 instructions per K=1024.
  Scheme D (pair+lo) costs 0.75x of fp32r and gives max rel err ~9e-3
  end-to-end (simulated); dropping the lo pass on either matrix is too
  imprecise (~0.18). Weights are pre-scaled by 32 so entries are ~N(0,1)
  (away from fp8 subnormals); the 1/32 is folded into activation scales.

Sharding: data-parallel over batch, one row per core (B == 8). Weights
replicated. kz/kh computed in [h-partition, s-free] layout; the recurrence is
a native tensor_tensor_scan along the free axis per (h-tile, s-block),
chained via the previous block's last column.

Engine placement per (s-block, h-tile): scalar computes c = sigmoid(-kz-bz),
a = sigmoid(kh+bh), m = kh+bh+0.5 (all fp16); DVE computes gt = max(a,m),
z = 1-c, and the scan; gpsimd (Pool) computes v = z*gt. This keeps every
engine under the PE's matmul time.

Host-side layout only (no math): x is fed pre-transposed and fp8-quantized
as (hi, lo) pairs; output comes back (H, S) fp16 and is transposed on host.
"""

import numpy as np

import concourse.bass as bass
import concourse.mybir as mybir
import concourse.tile as tile
from concourse import bacc
from concourse.bass_utils import run_bass_kernel_spmd

B, S, D, H = 8, 4096, 1024, 1024
N_CORES = 8
P = 128              # partitions
SB = 512             # s-block (columns per PSUM bank)
NSB = S // SB        # 8
DT = D // P          # 8 contraction tiles
KT = D // (2 * P)    # 4 double-row contraction tiles
HT = H // P          # 8 hidden tiles

F32 = mybir.dt.float32
F16 = mybir.dt.float16
F8 = mybir.dt.float8e4
MM_DT = F8           # referenced by test.py

# DR moving-block width (output columns per matmul instruction). The moving
# AP free size is 2*MMC.
MMC = 512

_CACHE = {}


def _build_program(ablate=(), repeat=1, bufs=None, sb=SB, mmc=MMC,
                   lo_pass=(True, True), v_engine="pool"):
    """ablate: subset of {'mm','act','dve','scan','outdma','xdma','pool'}.
    repeat: unroll the body N times (timing only). lo_pass: include the
    W_lo correction pass for (Wz, Wh). v_engine: 'pool' or 'dve'."""
    bufs = {**{"xin": 3, "psz": 2, "psh": 2, "inter": 3, "outp": 2},
            **(bufs or {})}
    nsb = S // sb
    nc = bacc.Bacc(trn_type="TRN2")

    xt8 = nc.dram_tensor("xt8", [D, 2 * S], F8, kind="ExternalInput")
    wz8 = nc.dram_tensor("wz8", [D, 2 * H], F8, kind="ExternalInput")
    wh8 = nc.dram_tensor("wh8", [D, 2 * H], F8, kind="ExternalInput")
    wzl8 = nc.dram_tensor("wzl8", [D, H], F8, kind="ExternalInput")
    whl8 = nc.dram_tensor("whl8", [D, H], F8, kind="ExternalInput")
    bzg = nc.dram_tensor("bzg", [P, HT], F32, kind="ExternalInput")
    bhg = nc.dram_tensor("bhg", [P, HT], F32, kind="ExternalInput")
    h0g = nc.dram_tensor("h0g", [P, HT], F32, kind="ExternalInput")
    hT = nc.dram_tensor("ht", [H, S], F16, kind="ExternalOutput")

    AF = mybir.ActivationFunctionType
    OP = mybir.AluOpType
    MPM = mybir.MatmulPerfMode.DoubleRow
    SC = 1.0 / 32.0

    with tile.TileContext(nc) as tc:
        with (
            tc.tile_pool(name="wpool", bufs=1) as wpool,
            tc.tile_pool(name="bias", bufs=1) as bias,
            tc.tile_pool(name="xin", bufs=bufs["xin"]) as xin,
            tc.tile_pool(name="psz", bufs=bufs["psz"], space="PSUM") as psz,
            tc.tile_pool(name="psh", bufs=bufs["psh"], space="PSUM") as psh,
            tc.tile_pool(name="inter", bufs=bufs["inter"]) as inter,
            tc.tile_pool(name="outp", bufs=bufs["outp"]) as outp,
        ):
            # Pair weights [p, dt, two, h]; chunked per dt so the first
            # matmuls can start before all weights land.
            wz_sb = wpool.tile([P, DT, 2, H], F8, tag="wz")
            wh_sb = wpool.tile([P, DT, 2, H], F8, tag="wh")
            wz_v = wz8.ap().rearrange("(dt p) (two h) -> p dt two h", p=P, two=2)
            wh_v = wh8.ap().rearrange("(dt p) (two h) -> p dt two h", p=P, two=2)
            for di in range(DT):
                nc.sync.dma_start(out=wz_sb[:, di:di + 1], in_=wz_v[:, di:di + 1])
                nc.sync.dma_start(out=wh_sb[:, di:di + 1], in_=wh_v[:, di:di + 1])
            # Lo-correction weights [p, kt, two, h] (slots = adjacent k-tiles)
            wzl_sb = wpool.tile([P, KT, 2, H], F8, tag="wzl")
            whl_sb = wpool.tile([P, KT, 2, H], F8, tag="whl")
            wzl_v = wzl8.ap().rearrange("(kt two p) h -> p kt two h", p=P, two=2)
            whl_v = whl8.ap().rearrange("(kt two p) h -> p kt two h", p=P, two=2)
            for ki in range(KT):
                nc.sync.dma_start(out=wzl_sb[:, ki:ki + 1], in_=wzl_v[:, ki:ki + 1])
                nc.sync.dma_start(out=whl_sb[:, ki:ki + 1], in_=whl_v[:, ki:ki + 1])

            # Bias / initial-state columns, [p(h-in-tile), h-tile]
            bz_sb = bias.tile([P, HT], F32, tag="bz")
            nc.sync.dma_start(out=bz_sb, in_=bzg.ap())
            bh_sb = bias.tile([P, HT], F32, tag="bh")
            nc.sync.dma_start(out=bh_sb, in_=bhg.ap())
            h0_sb = bias.tile([P, HT], F32, tag="h0")
            nc.sync.dma_start(out=h0_sb, in_=h0g.ap())

            nbz_sb = bias.tile([P, HT], F32, tag="nbz")
            nc.vector.tensor_scalar_mul(nbz_sb[:], bz_sb[:], -1.0)
            bhh_sb = bias.tile([P, HT], F32, tag="bhh")  # bh + 0.5
            nc.vector.tensor_scalar_add(bhh_sb[:], bh_sb[:], 0.5)

            # g0 = max(sigmoid(h0), h0 + 0.5)
            g0_s = bias.tile([P, HT], F32, tag="g0s")
            nc.scalar.activation(g0_s[:], h0_sb[:], AF.Sigmoid)
            g0_t = bias.tile([P, HT], F32, tag="g0t")
            nc.vector.tensor_scalar_add(g0_t[:], h0_sb[:], 0.5)
            g0 = bias.tile([P, HT], F32, tag="g0")
            nc.vector.tensor_max(g0[:], g0_s[:], g0_t[:])

            # x pairs are laid out [(dt p), (nsb two s)] so one s-block DMA
            # reads 2*sb contiguous columns per (dt, p) row.
            xT_v = xt8.ap().rearrange("(dt p) c -> p dt c", p=P)
            hT_v = hT.ap().rearrange("(ht p) s -> p ht s", p=P)

            nch = sb // mmc  # moving chunks per s-block

            for _rep in range(repeat):
              prev_out = [None] * HT
              for sbi in range(nsb):
                x_t = xin.tile([P, DT, 2, sb], F8, tag="x")
                if "xdma" not in ablate:
                    nc.sync.dma_start(
                        out=x_t,
                        in_=xT_v[:, :, sbi * 2 * sb:(sbi + 1) * 2 * sb],
                    )

                for hi in range(HT):
                    hs = slice(hi * P, (hi + 1) * P)
                    kz = psz.tile([P, sb], F32)
                    kh = psh.tile([P, sb], F32)
                    if "mm" not in ablate:
                        for ci in range(nch):
                            cs = slice(ci * mmc, (ci + 1) * mmc)
                            for w_sb, wl_sb, ps, lo in (
                                (wz_sb, wzl_sb, kz, lo_pass[0]),
                                (wh_sb, whl_sb, kh, lo_pass[1]),
                            ):
                                for di in range(DT):
                                    nc.tensor.matmul(
                                        ps[:, cs],
                                        w_sb[:, di, :, hs],
                                        x_t[:, di, :, cs],
                                        start=(di == 0),
                                        stop=(di == DT - 1 and not lo),
                                        perf_mode=MPM,
                                    )
                                if lo:
                                    for ki in range(KT):
                                        nc.tensor.matmul(
                                            ps[:, cs],
                                            wl_sb[:, ki, :, hs],
                                            x_t[:, 2 * ki:2 * ki + 2, 0, cs],
                                            start=False,
                                            stop=(ki == KT - 1),
                                            perf_mode=MPM,
                                        )

                    bc = slice(hi, hi + 1)
                    ct = inter.tile([P, sb], F16, tag="c")
                    at = inter.tile([P, sb], F16, tag="a")
                    mt = inter.tile([P, sb], F16, tag="m")
                    if "act" not in ablate:
                        # c = sigmoid(-(kz/32 + bz));  a = sigmoid(kh/32 + bh)
                        nc.scalar.activation(
                            ct[:], kz[:], AF.Sigmoid, bias=nbz_sb[:, bc],
                            scale=-SC,
                        )
                        nc.scalar.activation(
                            at[:], kh[:], AF.Sigmoid, bias=bh_sb[:, bc],
                            scale=SC,
                        )
                        # m = kh/32 + bh + 0.5
                        nc.scalar.activation(
                            mt[:], kh[:], AF.Identity, bias=bhh_sb[:, bc],
                            scale=SC,
                        )
                    tl = inter.tile([P, sb], F16, tag="tl")
                    zt = inter.tile([P, sb], F16, tag="z")
                    if "dve" not in ablate:
                        nc.vector.tensor_max(tl[:], at[:], mt[:])
                        # z = 1 - c  (tensor_scalar, 4x fp16 mode)
                        nc.vector.tensor_scalar(
                            zt[:], ct[:], -1.0, 1.0, OP.mult, OP.add
                        )
                    vt = inter.tile([P, sb], F16, tag="v")
                    if "pool" not in ablate:
                        eng = nc.gpsimd if v_engine == "pool" else nc.vector
                        eng.tensor_tensor(vt[:], zt[:], tl[:], OP.mult)

                    ot = outp.tile([P, sb], F16, tag=f"o{hi}")
                    if "scan" not in ablate:
                        init = (
                            g0[:, bc] if sbi == 0
                            else prev_out[hi][:, sb - 1:sb]
                        )
                        nc.vector.tensor_tensor_scan(
                            ot[:], ct[:], vt[:], init, op0=OP.mult, op1=OP.add
                        )
                        prev_out[hi] = ot
                    if "outdma" not in ablate:
                        nc.sync.dma_start(
                            out=hT_v[:, hi, sbi * sb:(sbi + 1) * sb], in_=ot[:]
                        )
    nc.finalize()
    return nc


def _get_program():
    if "nc" not in _CACHE:
        _CACHE["nc"] = _build_program()
    return _CACHE["nc"]


def _q8(v):
    return v.astype(mybir.dt.np(F8))


def prep_core_inputs(x_b, h0_b, consts):
    """Per-core input map for batch row x_b (S, D), h0_b (H,)."""
    xT = np.ascontiguousarray(x_b.T).astype(np.float32)      # (D, S)
    xh = _q8(xT)
    xl = _q8(xT - xh.astype(np.float32))
    # (d, (nsb two s)): per s-block, the hi block then the lo block
    xt8 = np.stack(
        [xh.reshape(D, NSB, SB), xl.reshape(D, NSB, SB)], axis=2
    ).reshape(D, 2 * S)
    return {
        "xt8": xt8,
        "h0g": np.ascontiguousarray(h0_b.reshape(HT, P).T),
        **consts,
    }


def prep_const_inputs(Wz, bz, Wh, bh):
    """Weight/bias tensors shared by all cores."""
    consts = {}
    for name, W in (("z", Wz), ("h", Wh)):
        W32T = np.ascontiguousarray(W.T).astype(np.float32) * 32.0  # (D, H)
        Whi = _q8(W32T)
        Wlo = _q8(W32T - Whi.astype(np.float32))
        consts[f"w{name}8"] = np.stack([Whi, Whi], axis=1).reshape(D, 2 * H)
        consts[f"w{name}l8"] = Wlo
    consts["bzg"] = np.ascontiguousarray(bz.reshape(HT, P).T)
    consts["bhg"] = np.ascontiguousarray(bh.reshape(HT, P).T)
    return consts


def run(inputs, **kw):
    """Run on hardware; returns (output (B,S,H) fp32, BassKernelResults)."""
    x = np.asarray(inputs["x"], dtype=np.float32)
    h0 = np.asarray(inputs["h0"], dtype=np.float32)
    consts = prep_const_inputs(
        np.asarray(inputs["Wz"], dtype=np.float32),
        np.asarray(inputs["bz"], dtype=np.float32),
        np.asarray(inputs["Wh"], dtype=np.float32),
        np.asarray(inputs["bh"], dtype=np.float32),
    )
    in_maps = [
        prep_core_inputs(x[b], h0[b, 0], consts) for b in range(N_CORES)
    ]

    nc = _get_program()
    res = run_bass_kernel_spmd(nc, in_maps, core_ids=list(range(N_CORES)), **kw)
    out = np.stack(
        [res.results[b]["ht"].astype(np.float32).T for b in range(N_CORES)],
        axis=0,
    )
    return np.ascontiguousarray(out), res


def kernel(**inputs):
    out, _ = run(inputs)
    return out


# revision 9
# speedup vs baseline: 1.3075x; 1.3075x over previous
"""MinGRU cell on 8 Trainium2 NeuronCores.

Math: per (batch b, hidden channel j), the reference computes (in log space)
the linear recurrence

    h_t = c_t * h_{t-1} + v_t,      h_0 = g(h0)
    c_t = 1 - sigmoid(kz_t) = sigmoid(-kz_t)
    v_t = z_t * g(kh_t),  z_t = 1 - c_t
    kz = x @ Wz^T + bz,  kh = x @ Wh^T + bh
    g(u) = max(sigmoid(u), u + 0.5)   (exact identity for the reference's g)

All quantities are positive and O(1), so the linear-space recurrence in fp32
with fp16 intermediates is accurate to ~2e-3 (verified vs the log-space
reference; tolerance is 2e-2).

Matmul styles (MM_STYLE):
  'f32r' - single-pass float32r matmuls (8 instr/tile/matrix, ~exact).
  'kd'   - fp8(e4m3) DoubleRow, scheme D as 3 K=256 passes (x_hi*W_hi +
           x_lo*W_hi + x_hi*W_lo), 12 instr/tile/matrix, max rel ~9.7e-3.
  'pair' - scheme D with hi/lo slot-pair passes + K-doubled lo pass.
Weights for fp8 styles are pre-scaled by 32 (entries ~N(0,1), away from fp8
subnormals); the 1/32 is folded into the activation scales.

Sharding: data-parallel over batch, one row per core (B == 8). Weights
replicated. kz/kh computed in [h-partition, s-free] layout; the recurrence is
a native tensor_tensor_scan along the free axis per (h-tile, s-block),
chained via the previous block's last column.

Engine placement per (s-block, h-tile): scalar computes c = sigmoid(-kz-bz),
a = sigmoid(kh+bh), m = kh+bh+0.5 (all fp16); DVE computes gt = max(a,m),
z = 1-c, and the scan; gpsimd (Pool) computes v = z*gt. This keeps every
engine under the PE's matmul time.

Host-side layout only (no math): x is fed pre-transposed (and fp8-quantized
as (hi, lo) blocks for fp8 styles); output comes back (H, S) fp16 and is
transposed on host.
"""

import numpy as np

import concourse.bass as bass
import concourse.mybir as mybir
import concourse.tile as tile
from concourse import bacc
from concourse.bass_utils import run_bass_kernel_spmd

B, S, D, H = 8, 4096, 1024, 1024
N_CORES = 8
P = 128              # partitions
SB = 512             # s-block (columns per PSUM bank)
NSB = S // SB        # 8
DT = D // P          # 8 contraction tiles
KT = D // (2 * P)    # 4 double-row contraction tiles
HT = H // P          # 8 hidden tiles

F32 = mybir.dt.float32
F16 = mybir.dt.float16
F8 = mybir.dt.float8e4
F32R = mybir.dt.float32r
MM_DT = F8           # referenced by test.py

MM_STYLE = "f32r"    # 'f32r' | 'kd' | 'pair'

# DR moving-block width (output columns per matmul instruction).
MMC = 512

_CACHE = {}


def _build_program(ablate=(), repeat=1, bufs=None, sb=SB, mmc=MMC,
                   lo_pass=(True, True), v_engine="pool", mm_style=None):
    """ablate: subset of {'mm','mmlite','act','dve','scan','outdma','xdma',
    'pool'}. repeat: unroll the body N times (timing only). lo_pass: include
    the W_lo correction pass per matrix (fp8 styles). v_engine: 'pool'|'dve'.
    """
    if mm_style is None:
        mm_style = MM_STYLE
    bufs = {**{"xin": 3, "psz": 2, "psh": 2, "inter": 3, "outp": 2},
            **(bufs or {})}
    nsb = S // sb
    fp8 = mm_style != "f32r"
    nc = bacc.Bacc(trn_type="TRN2")

    if fp8:
        xt8 = nc.dram_tensor("xt8", [D, 2 * S], F8, kind="ExternalInput")
        wcols = H if mm_style == "kd" else 2 * H
        wz8 = nc.dram_tensor("wz8", [D, wcols], F8, kind="ExternalInput")
        wh8 = nc.dram_tensor("wh8", [D, wcols], F8, kind="ExternalInput")
        wzl8 = nc.dram_tensor("wzl8", [D, H], F8, kind="ExternalInput")
        whl8 = nc.dram_tensor("whl8", [D, H], F8, kind="ExternalInput")
    else:
        xt8 = nc.dram_tensor("xt8", [D, S], F32R, kind="ExternalInput")
        wz8 = nc.dram_tensor("wz8", [D, H], F32R, kind="ExternalInput")
        wh8 = nc.dram_tensor("wh8", [D, H], F32R, kind="ExternalInput")
    bzg = nc.dram_tensor("bzg", [P, HT], F32, kind="ExternalInput")
    bhg = nc.dram_tensor("bhg", [P, HT], F32, kind="ExternalInput")
    h0g = nc.dram_tensor("h0g", [P, HT], F32, kind="ExternalInput")
    hT = nc.dram_tensor("ht", [H, S], F16, kind="ExternalOutput")

    AF = mybir.ActivationFunctionType
    OP = mybir.AluOpType
    MPM = mybir.MatmulPerfMode.DoubleRow
    SC = 1.0 / 32.0 if fp8 else 1.0

    with tile.TileContext(nc) as tc:
        with (
            tc.tile_pool(name="wpool", bufs=1) as wpool,
            tc.tile_pool(name="bias", bufs=1) as bias,
            tc.tile_pool(name="xin", bufs=bufs["xin"]) as xin,
            tc.tile_pool(name="psz", bufs=bufs["psz"], space="PSUM") as psz,
            tc.tile_pool(name="psh", bufs=bufs["psh"], space="PSUM") as psh,
            tc.tile_pool(name="inter", bufs=bufs["inter"]) as inter,
            tc.tile_pool(name="outp", bufs=bufs["outp"]) as outp,
        ):
            wzl_sb = whl_sb = None
            if mm_style == "f32r":
                wz_sb = wpool.tile([P, DT, H], F32R, tag="wz")
                wh_sb = wpool.tile([P, DT, H], F32R, tag="wh")
                wz_v = wz8.ap().rearrange("(dt p) h -> p dt h", p=P)
                wh_v = wh8.ap().rearrange("(dt p) h -> p dt h", p=P)
                for di in range(DT):
                    nc.sync.dma_start(out=wz_sb[:, di:di + 1],
                                      in_=wz_v[:, di:di + 1])
                    nc.sync.dma_start(out=wh_sb[:, di:di + 1],
                                      in_=wh_v[:, di:di + 1])
            elif mm_style == "kd":
                # Hi weights in k-paired layout [p, kt, two, h] (slots are
                # adjacent k-tiles; natural d-order on the host side).
                wz_sb = wpool.tile([P, KT, 2, H], F8, tag="wz")
                wh_sb = wpool.tile([P, KT, 2, H], F8, tag="wh")
                wz_v = wz8.ap().rearrange("(kt two p) h -> p kt two h",
                                          p=P, two=2)
                wh_v = wh8.ap().rearrange("(kt two p) h -> p kt two h",
                                          p=P, two=2)
                for ki in range(KT):
                    nc.sync.dma_start(out=wz_sb[:, ki:ki + 1],
                                      in_=wz_v[:, ki:ki + 1])
                    nc.sync.dma_start(out=wh_sb[:, ki:ki + 1],
                                      in_=wh_v[:, ki:ki + 1])
            else:
                # Pair weights [p, dt, two, h], hi duplicated in both slots.
                wz_sb = wpool.tile([P, DT, 2, H], F8, tag="wz")
                wh_sb = wpool.tile([P, DT, 2, H], F8, tag="wh")
                wz_v = wz8.ap().rearrange("(dt p) (two h) -> p dt two h",
                                          p=P, two=2)
                wh_v = wh8.ap().rearrange("(dt p) (two h) -> p dt two h",
                                          p=P, two=2)
                for di in range(DT):
                    nc.sync.dma_start(out=wz_sb[:, di:di + 1],
                                      in_=wz_v[:, di:di + 1])
                    nc.sync.dma_start(out=wh_sb[:, di:di + 1],
                                      in_=wh_v[:, di:di + 1])
            if fp8:
                # Lo-correction weights [p, kt, two, h]
                wzl_sb = wpool.tile([P, KT, 2, H], F8, tag="wzl")
                whl_sb = wpool.tile([P, KT, 2, H], F8, tag="whl")
                wzl_v = wzl8.ap().rearrange("(kt two p) h -> p kt two h",
                                            p=P, two=2)
                whl_v = whl8.ap().rearrange("(kt two p) h -> p kt two h",
                                            p=P, two=2)
                for ki in range(KT):
                    nc.sync.dma_start(out=wzl_sb[:, ki:ki + 1],
                                      in_=wzl_v[:, ki:ki + 1])
                    nc.sync.dma_start(out=whl_sb[:, ki:ki + 1],
                                      in_=whl_v[:, ki:ki + 1])

            # Bias / initial-state columns, [p(h-in-tile), h-tile]
            bz_sb = bias.tile([P, HT], F32, tag="bz")
            nc.sync.dma_start(out=bz_sb, in_=bzg.ap())
            bh_sb = bias.tile([P, HT], F32, tag="bh")
            nc.sync.dma_start(out=bh_sb, in_=bhg.ap())
            h0_sb = bias.tile([P, HT], F32, tag="h0")
            nc.sync.dma_start(out=h0_sb, in_=h0g.ap())

            nbz_sb = bias.tile([P, HT], F32, tag="nbz")
            nc.vector.tensor_scalar_mul(nbz_sb[:], bz_sb[:], -1.0)
            bhh_sb = bias.tile([P, HT], F32, tag="bhh")  # bh + 0.5
            nc.vector.tensor_scalar_add(bhh_sb[:], bh_sb[:], 0.5)

            # g0 = max(sigmoid(h0), h0 + 0.5)
            g0_s = bias.tile([P, HT], F32, tag="g0s")
            nc.scalar.activation(g0_s[:], h0_sb[:], AF.Sigmoid)
            g0_t = bias.tile([P, HT], F32, tag="g0t")
            nc.vector.tensor_scalar_add(g0_t[:], h0_sb[:], 0.5)
            g0 = bias.tile([P, HT], F32, tag="g0")
            nc.vector.tensor_max(g0[:], g0_s[:], g0_t[:])

            # x layout: fp8 -> [(dt p), (nsb two s)] (hi block then lo block
            # per s-block, 1KB contiguous runs); f32r -> [(dt p), s].
            xT_v = xt8.ap().rearrange("(dt p) c -> p dt c", p=P)
            hT_v = hT.ap().rearrange("(ht p) s -> p ht s", p=P)

            nch = sb // mmc  # moving chunks per s-block

            for _rep in range(repeat):
              prev_out = [None] * HT
              for sbi in range(nsb):
                if fp8:
                    x_t = xin.tile([P, DT, 2, sb], F8, tag="x")
                    xsrc = xT_v[:, :, sbi * 2 * sb:(sbi + 1) * 2 * sb]
                else:
                    x_t = xin.tile([P, DT, sb], F32R, tag="x")
                    xsrc = xT_v[:, :, sbi * sb:(sbi + 1) * sb]
                if "xdma" not in ablate:
                    nc.sync.dma_start(out=x_t, in_=xsrc)

                for hi in range(HT):
                    hs = slice(hi * P, (hi + 1) * P)
                    kz = psz.tile([P, sb], F32)
                    kh = psh.tile([P, sb], F32)
                    if "mmlite" in ablate:
                        # one matmul per psum tile: keeps dataflow deps for
                        # elementwise timing at a fraction of the PE work
                        for ps in (kz, kh):
                            if mm_style == "f32r":
                                nc.tensor.matmul(
                                    ps[:, 0:sb], wz_sb[:, 0, hs],
                                    x_t[:, 0, 0:sb], start=True, stop=True,
                                )
                            elif mm_style == "kd":
                                nc.tensor.matmul(
                                    ps[:, 0:sb], wz_sb[:, 0, :, hs],
                                    x_t[:, 0:2, 0, 0:sb],
                                    start=True, stop=True, perf_mode=MPM,
                                )
                            else:
                                nc.tensor.matmul(
                                    ps[:, 0:sb], wz_sb[:, 0, :, hs],
                                    x_t[:, 0, :, 0:sb],
                                    start=True, stop=True, perf_mode=MPM,
                                )
                    elif "mm" not in ablate:
                        for ci in range(nch):
                            cs = slice(ci * mmc, (ci + 1) * mmc)
                            for w_sb, wl_sb, ps, lo in (
                                (wz_sb, wzl_sb, kz, lo_pass[0]),
                                (wh_sb, whl_sb, kh, lo_pass[1]),
                            ):
                                if mm_style == "f32r":
                                    for di in range(DT):
                                        nc.tensor.matmul(
                                            ps[:, cs],
                                            w_sb[:, di, hs],
                                            x_t[:, di, cs],
                                            start=(di == 0),
                                            stop=(di == DT - 1),
                                        )
                                elif mm_style == "kd":
                                    # 3 K=256 passes: xh*Whi, xl*Whi, xh*Wlo
                                    passes = [(w_sb, 0), (w_sb, 1)]
                                    if lo:
                                        passes.append((wl_sb, 0))
                                    for pi, (wt, slot) in enumerate(passes):
                                        for ki in range(KT):
                                            nc.tensor.matmul(
                                                ps[:, cs],
                                                wt[:, ki, :, hs],
                                                x_t[:, 2 * ki:2 * ki + 2,
                                                    slot, cs],
                                                start=(pi == 0 and ki == 0),
                                                stop=(pi == len(passes) - 1
                                                      and ki == KT - 1),
                                                perf_mode=MPM,
                                            )
                                else:
                                    for di in range(DT):
                                        nc.tensor.matmul(
                                            ps[:, cs],
                                            w_sb[:, di, :, hs],
                                            x_t[:, di, :, cs],
                                            start=(di == 0),
                                            stop=(di == DT - 1 and not lo),
                                            perf_mode=MPM,
                                        )
                                    if lo:
                                        for ki in range(KT):
                                            nc.tensor.matmul(
                                                ps[:, cs],
                                                wl_sb[:, ki, :, hs],
                                                x_t[:, 2 * ki:2 * ki + 2,
                                                    0, cs],
                                                start=False,
                                                stop=(ki == KT - 1),
                                                perf_mode=MPM,
                                            )

                    bc = slice(hi, hi + 1)
                    ct = inter.tile([P, sb], F16, tag="c")
                    at = inter.tile([P, sb], F16, tag="a")
                    mt = inter.tile([P, sb], F16, tag="m")
                    if "act" not in ablate:
                        # c = sigmoid(-(kz*SC + bz));  a = sigmoid(kh*SC + bh)
                        nc.scalar.activation(
                            ct[:], kz[:], AF.Sigmoid, bias=nbz_sb[:, bc],
                            scale=-SC,
                        )
                        nc.scalar.activation(
                            at[:], kh[:], AF.Sigmoid, bias=bh_sb[:, bc],
                            scale=SC,
                        )
                        # m = kh*SC + bh + 0.5
                        nc.scalar.activation(
                            mt[:], kh[:], AF.Identity, bias=bhh_sb[:, bc],
                            scale=SC,
                        )
                    tl = inter.tile([P, sb], F16, tag="tl")
                    zt = inter.tile([P, sb], F16, tag="z")
                    if "dve" not in ablate:
                        nc.vector.tensor_max(tl[:], at[:], mt[:])
                        # z = 1 - c  (tensor_scalar, 4x fp16 mode)
                        nc.vector.tensor_scalar(
                            zt[:], ct[:], -1.0, 1.0, OP.mult, OP.add
                        )
                    vt = inter.tile([P, sb], F16, tag="v")
                    if "pool" not in ablate:
                        eng = nc.gpsimd if v_engine == "pool" else nc.vector
                        eng.tensor_tensor(vt[:], zt[:], tl[:], OP.mult)

                    ot = outp.tile([P, sb], F16, tag=f"o{hi}")
                    if "scan" not in ablate:
                        init = (
                            g0[:, bc] if sbi == 0
                            else prev_out[hi][:, sb - 1:sb]
                        )
                        nc.vector.tensor_tensor_scan(
                            ot[:], ct[:], vt[:], init, op0=OP.mult, op1=OP.add
                        )
                        prev_out[hi] = ot
                    if "outdma" not in ablate:
                        nc.sync.dma_start(
                            out=hT_v[:, hi, sbi * sb:(sbi + 1) * sb], in_=ot[:]
                        )
    nc.finalize()
    return nc


def _get_program():
    if "nc" not in _CACHE:
        _CACHE["nc"] = _build_program()
    return _CACHE["nc"]


def _q8(v):
    return v.astype(mybir.dt.np(F8))


def prep_core_inputs(x_b, h0_b, consts, mm_style=None):
    """Per-core input map for batch row x_b (S, D), h0_b (H,)."""
    if mm_style is None:
        mm_style = MM_STYLE
    xT = np.ascontiguousarray(x_b.T).astype(np.float32)      # (D, S)
    if mm_style == "f32r":
        xt8 = xT
    else:
        xh = _q8(xT)
        xl = _q8(xT - xh.astype(np.float32))
        # (d, (nsb two s)): per s-block, the hi block then the lo block
        xt8 = np.stack(
            [xh.reshape(D, NSB, SB), xl.reshape(D, NSB, SB)], axis=2
        ).reshape(D, 2 * S)
    return {
        "xt8": xt8,
        "h0g": np.ascontiguousarray(h0_b.reshape(HT, P).T),
        **consts,
    }


def prep_const_inputs(Wz, bz, Wh, bh, mm_style=None):
    """Weight/bias tensors shared by all cores."""
    if mm_style is None:
        mm_style = MM_STYLE
    consts = {}
    for name, W in (("z", Wz), ("h", Wh)):
        WT = np.ascontiguousarray(W.T).astype(np.float32)  # (D, H)
        if mm_style == "f32r":
            consts[f"w{name}8"] = WT
            continue
        W32T = WT * 32.0
        Whi = _q8(W32T)
        Wlo = _q8(W32T - Whi.astype(np.float32))
        if mm_style == "kd":
            consts[f"w{name}8"] = Whi
        else:
            consts[f"w{name}8"] = np.stack([Whi, Whi], axis=1).reshape(D, 2 * H)
        consts[f"w{name}l8"] = Wlo
    consts["bzg"] = np.ascontiguousarray(bz.reshape(HT, P).T)
    consts["bhg"] = np.ascontiguousarray(bh.reshape(HT, P).T)
    return consts


def run(inputs, **kw):
    """Run on hardware; returns (output (B,S,H) fp32, BassKernelResults)."""
    x = np.asarray(inputs["x"], dtype=np.float32)
    h0 = np.asarray(inputs["h0"], dtype=np.float32)
    consts = prep_const_inputs(
        np.asarray(inputs["Wz"], dtype=np.float32),
        np.asarray(inputs["bz"], dtype=np.float32),
        np.asarray(inputs["Wh"], dtype=np.float32),
        np.asarray(inputs["bh"], dtype=np.float32),
    )
    in_maps = [
        prep_core_inputs(x[b], h0[b, 0], consts) for b in range(N_CORES)
    ]

    nc = _get_program()
    res = run_bass_kernel_spmd(nc, in_maps, core_ids=list(range(N_CORES)), **kw)
    out = np.stack(
        [res.results[b]["ht"].astype(np.float32).T for b in range(N_CORES)],
        axis=0,
    )
    return np.ascontiguousarray(out), res


def kernel(**inputs):
    out, _ = run(inputs)
    return out


# revision 10
# speedup vs baseline: 3.0046x; 2.2980x over previous
"""MinGRU cell on 8 Trainium2 NeuronCores.

Math: per (batch b, hidden channel j), the reference computes (in log space)
the linear recurrence

    h_t = c_t * h_{t-1} + v_t,      h_0 = g(h0)
    c_t = 1 - sigmoid(kz_t) = sigmoid(-kz_t)
    v_t = z_t * g(kh_t),  z_t = 1 - c_t
    kz = x @ Wz^T + bz,  kh = x @ Wh^T + bh
    g(u) = max(sigmoid(u), u + 0.5)   (exact identity for the reference's g)

All quantities are positive and O(1), so the linear-space recurrence in fp32
with fp16 intermediates is accurate to ~2e-3 (verified vs the log-space
reference; tolerance is 2e-2).

Matmul styles (MM_STYLE):
  'f32r' - single-pass float32r matmuls (8 instr/tile/matrix, ~exact).
  'kd'   - fp8(e4m3) DoubleRow, scheme D as 3 K=256 passes (x_hi*W_hi +
           x_lo*W_hi + x_hi*W_lo), 12 instr/tile/matrix, max rel ~9.7e-3.
  'pair' - scheme D with hi/lo slot-pair passes + K-doubled lo pass.
Weights for fp8 styles are pre-scaled by 32 (entries ~N(0,1), away from fp8
subnormals); the 1/32 is folded into the activation scales.

Sharding: data-parallel over batch, one row per core (B == 8). Weights
replicated. kz/kh computed in [h-partition, s-free] layout; the recurrence is
a native tensor_tensor_scan along the free axis per (h-tile, s-block),
chained via the previous block's last column.

Engine placement per (s-block, h-tile): scalar computes c = sigmoid(-kz-bz),
a = sigmoid(kh+bh), m = kh+bh+0.5 (all fp16); DVE computes gt = max(a,m),
z = 1-c, v = z*gt, and the scan. Measured: the whole elementwise pipeline is
~61us/iter, far under the PE matmul time, so placement is not critical --
but gpsimd (v_engine='pool') measured ~4us/tile and must not be used.

Host-side layout only (no math): x is fed pre-transposed (and fp8-quantized
as (hi, lo) blocks for fp8 styles); output comes back (H, S) fp16 and is
transposed on host.
"""

import numpy as np

import concourse.bass as bass
import concourse.mybir as mybir
import concourse.tile as tile
from concourse import bacc
from concourse.bass_utils import run_bass_kernel_spmd

B, S, D, H = 8, 4096, 1024, 1024
N_CORES = 8
P = 128              # partitions
SB = 512             # s-block (columns per PSUM bank)
NSB = S // SB        # 8
DT = D // P          # 8 contraction tiles
KT = D // (2 * P)    # 4 double-row contraction tiles
HT = H // P          # 8 hidden tiles

F32 = mybir.dt.float32
F16 = mybir.dt.float16
F8 = mybir.dt.float8e4
F32R = mybir.dt.float32r
MM_DT = F8           # referenced by test.py

MM_STYLE = "f32r"    # 'f32r' | 'kd' | 'pair'

# DR moving-block width (output columns per matmul instruction).
MMC = 512

_CACHE = {}


def _build_program(ablate=(), repeat=1, bufs=None, sb=SB, mmc=MMC,
                   lo_pass=(True, True), v_engine="dve", mm_style=None):
    """ablate: subset of {'mm','mmlite','act','dve','scan','outdma','xdma',
    'pool'}. repeat: unroll the body N times (timing only). lo_pass: include
    the W_lo correction pass per matrix (fp8 styles). v_engine: 'pool'|'dve'.
    """
    if mm_style is None:
        mm_style = MM_STYLE
    bufs = {**{"xin": 3, "psz": 2, "psh": 2, "inter": 3, "outp": 2},
            **(bufs or {})}
    nsb = S // sb
    fp8 = mm_style != "f32r"
    nc = bacc.Bacc(trn_type="TRN2")

    if fp8:
        xt8 = nc.dram_tensor("xt8", [D, 2 * S], F8, kind="ExternalInput")
        wcols = H if mm_style == "kd" else 2 * H
        wz8 = nc.dram_tensor("wz8", [D, wcols], F8, kind="ExternalInput")
        wh8 = nc.dram_tensor("wh8", [D, wcols], F8, kind="ExternalInput")
        wzl8 = nc.dram_tensor("wzl8", [D, H], F8, kind="ExternalInput")
        whl8 = nc.dram_tensor("whl8", [D, H], F8, kind="ExternalInput")
    else:
        xt8 = nc.dram_tensor("xt8", [D, S], F32R, kind="ExternalInput")
        wz8 = nc.dram_tensor("wz8", [D, H], F32R, kind="ExternalInput")
        wh8 = nc.dram_tensor("wh8", [D, H], F32R, kind="ExternalInput")
    bzg = nc.dram_tensor("bzg", [P, HT], F32, kind="ExternalInput")
    bhg = nc.dram_tensor("bhg", [P, HT], F32, kind="ExternalInput")
    h0g = nc.dram_tensor("h0g", [P, HT], F32, kind="ExternalInput")
    hT = nc.dram_tensor("ht", [H, S], F16, kind="ExternalOutput")

    AF = mybir.ActivationFunctionType
    OP = mybir.AluOpType
    MPM = mybir.MatmulPerfMode.DoubleRow
    SC = 1.0 / 32.0 if fp8 else 1.0

    with tile.TileContext(nc) as tc:
        with (
            tc.tile_pool(name="wpool", bufs=1) as wpool,
            tc.tile_pool(name="bias", bufs=1) as bias,
            tc.tile_pool(name="xin", bufs=bufs["xin"]) as xin,
            tc.tile_pool(name="psz", bufs=bufs["psz"], space="PSUM") as psz,
            tc.tile_pool(name="psh", bufs=bufs["psh"], space="PSUM") as psh,
            tc.tile_pool(name="inter", bufs=bufs["inter"]) as inter,
            tc.tile_pool(name="outp", bufs=bufs["outp"]) as outp,
        ):
            wzl_sb = whl_sb = None
            if mm_style == "f32r":
                wz_sb = wpool.tile([P, DT, H], F32R, tag="wz")
                wh_sb = wpool.tile([P, DT, H], F32R, tag="wh")
                wz_v = wz8.ap().rearrange("(dt p) h -> p dt h", p=P)
                wh_v = wh8.ap().rearrange("(dt p) h -> p dt h", p=P)
                for di in range(DT):
                    nc.sync.dma_start(out=wz_sb[:, di:di + 1],
                                      in_=wz_v[:, di:di + 1])
                    nc.sync.dma_start(out=wh_sb[:, di:di + 1],
                                      in_=wh_v[:, di:di + 1])
            elif mm_style == "kd":
                # Hi weights in k-paired layout [p, kt, two, h] (slots are
                # adjacent k-tiles; natural d-order on the host side).
                wz_sb = wpool.tile([P, KT, 2, H], F8, tag="wz")
                wh_sb = wpool.tile([P, KT, 2, H], F8, tag="wh")
                wz_v = wz8.ap().rearrange("(kt two p) h -> p kt two h",
                                          p=P, two=2)
                wh_v = wh8.ap().rearrange("(kt two p) h -> p kt two h",
                                          p=P, two=2)
                for ki in range(KT):
                    nc.sync.dma_start(out=wz_sb[:, ki:ki + 1],
                                      in_=wz_v[:, ki:ki + 1])
                    nc.sync.dma_start(out=wh_sb[:, ki:ki + 1],
                                      in_=wh_v[:, ki:ki + 1])
            else:
                # Pair weights [p, dt, two, h], hi duplicated in both slots.
                wz_sb = wpool.tile([P, DT, 2, H], F8, tag="wz")
                wh_sb = wpool.tile([P, DT, 2, H], F8, tag="wh")
                wz_v = wz8.ap().rearrange("(dt p) (two h) -> p dt two h",
                                          p=P, two=2)
                wh_v = wh8.ap().rearrange("(dt p) (two h) -> p dt two h",
                                          p=P, two=2)
                for di in range(DT):
                    nc.sync.dma_start(out=wz_sb[:, di:di + 1],
                                      in_=wz_v[:, di:di + 1])
                    nc.sync.dma_start(out=wh_sb[:, di:di + 1],
                                      in_=wh_v[:, di:di + 1])
            if fp8:
                # Lo-correction weights [p, kt, two, h]
                wzl_sb = wpool.tile([P, KT, 2, H], F8, tag="wzl")
                whl_sb = wpool.tile([P, KT, 2, H], F8, tag="whl")
                wzl_v = wzl8.ap().rearrange("(kt two p) h -> p kt two h",
                                            p=P, two=2)
                whl_v = whl8.ap().rearrange("(kt two p) h -> p kt two h",
                                            p=P, two=2)
                for ki in range(KT):
                    nc.sync.dma_start(out=wzl_sb[:, ki:ki + 1],
                                      in_=wzl_v[:, ki:ki + 1])
                    nc.sync.dma_start(out=whl_sb[:, ki:ki + 1],
                                      in_=whl_v[:, ki:ki + 1])

            # Bias / initial-state columns, [p(h-in-tile), h-tile]
            bz_sb = bias.tile([P, HT], F32, tag="bz")
            nc.sync.dma_start(out=bz_sb, in_=bzg.ap())
            bh_sb = bias.tile([P, HT], F32, tag="bh")
            nc.sync.dma_start(out=bh_sb, in_=bhg.ap())
            h0_sb = bias.tile([P, HT], F32, tag="h0")
            nc.sync.dma_start(out=h0_sb, in_=h0g.ap())

            nbz_sb = bias.tile([P, HT], F32, tag="nbz")
            nc.vector.tensor_scalar_mul(nbz_sb[:], bz_sb[:], -1.0)
            bhh_sb = bias.tile([P, HT], F32, tag="bhh")  # bh + 0.5
            nc.vector.tensor_scalar_add(bhh_sb[:], bh_sb[:], 0.5)

            # g0 = max(sigmoid(h0), h0 + 0.5)
            g0_s = bias.tile([P, HT], F32, tag="g0s")
            nc.scalar.activation(g0_s[:], h0_sb[:], AF.Sigmoid)
            g0_t = bias.tile([P, HT], F32, tag="g0t")
            nc.vector.tensor_scalar_add(g0_t[:], h0_sb[:], 0.5)
            g0 = bias.tile([P, HT], F32, tag="g0")
            nc.vector.tensor_max(g0[:], g0_s[:], g0_t[:])

            # x layout: fp8 -> [(dt p), (nsb two s)] (hi block then lo block
            # per s-block, 1KB contiguous runs); f32r -> [(dt p), s].
            xT_v = xt8.ap().rearrange("(dt p) c -> p dt c", p=P)
            hT_v = hT.ap().rearrange("(ht p) s -> p ht s", p=P)

            nch = sb // mmc  # moving chunks per s-block

            for _rep in range(repeat):
              prev_out = [None] * HT
              for sbi in range(nsb):
                if fp8:
                    x_t = xin.tile([P, DT, 2, sb], F8, tag="x")
                    xsrc = xT_v[:, :, sbi * 2 * sb:(sbi + 1) * 2 * sb]
                else:
                    x_t = xin.tile([P, DT, sb], F32R, tag="x")
                    xsrc = xT_v[:, :, sbi * sb:(sbi + 1) * sb]
                if "xdma" not in ablate:
                    nc.sync.dma_start(out=x_t, in_=xsrc)

                for hi in range(HT):
                    hs = slice(hi * P, (hi + 1) * P)
                    kz = psz.tile([P, sb], F32)
                    kh = psh.tile([P, sb], F32)
                    if "mmlite" in ablate:
                        # one matmul per psum tile: keeps dataflow deps for
                        # elementwise timing at a fraction of the PE work
                        for ps in (kz, kh):
                            if mm_style == "f32r":
                                nc.tensor.matmul(
                                    ps[:, 0:sb], wz_sb[:, 0, hs],
                                    x_t[:, 0, 0:sb], start=True, stop=True,
                                )
                            elif mm_style == "kd":
                                nc.tensor.matmul(
                                    ps[:, 0:sb], wz_sb[:, 0, :, hs],
                                    x_t[:, 0:2, 0, 0:sb],
                                    start=True, stop=True, perf_mode=MPM,
                                )
                            else:
                                nc.tensor.matmul(
                                    ps[:, 0:sb], wz_sb[:, 0, :, hs],
                                    x_t[:, 0, :, 0:sb],
                                    start=True, stop=True, perf_mode=MPM,
                                )
                    elif "mm" not in ablate:
                        for ci in range(nch):
                            cs = slice(ci * mmc, (ci + 1) * mmc)
                            for w_sb, wl_sb, ps, lo in (
                                (wz_sb, wzl_sb, kz, lo_pass[0]),
                                (wh_sb, whl_sb, kh, lo_pass[1]),
                            ):
                                if mm_style == "f32r":
                                    for di in range(DT):
                                        nc.tensor.matmul(
                                            ps[:, cs],
                                            w_sb[:, di, hs],
                                            x_t[:, di, cs],
                                            start=(di == 0),
                                            stop=(di == DT - 1),
                                        )
                                elif mm_style == "kd":
                                    # 3 K=256 passes: xh*Whi, xl*Whi, xh*Wlo
                                    passes = [(w_sb, 0), (w_sb, 1)]
                                    if lo:
                                        passes.append((wl_sb, 0))
                                    for pi, (wt, slot) in enumerate(passes):
                                        for ki in range(KT):
                                            nc.tensor.matmul(
                                                ps[:, cs],
                                                wt[:, ki, :, hs],
                                                x_t[:, 2 * ki:2 * ki + 2,
                                                    slot, cs],
                                                start=(pi == 0 and ki == 0),
                                                stop=(pi == len(passes) - 1
                                                      and ki == KT - 1),
                                                perf_mode=MPM,
                                            )
                                else:
                                    for di in range(DT):
                                        nc.tensor.matmul(
                                            ps[:, cs],
                                            w_sb[:, di, :, hs],
                                            x_t[:, di, :, cs],
                                            start=(di == 0),
                                            stop=(di == DT - 1 and not lo),
                                            perf_mode=MPM,
                                        )
                                    if lo:
                                        for ki in range(KT):
                                            nc.tensor.matmul(
                                                ps[:, cs],
                                                wl_sb[:, ki, :, hs],
                                                x_t[:, 2 * ki:2 * ki + 2,
                                                    0, cs],
                                                start=False,
                                                stop=(ki == KT - 1),
                                                perf_mode=MPM,
                                            )

                    bc = slice(hi, hi + 1)
                    ct = inter.tile([P, sb], F16, tag="c")
                    at = inter.tile([P, sb], F16, tag="a")
                    mt = inter.tile([P, sb], F16, tag="m")
                    if "act" not in ablate:
                        # c = sigmoid(-(kz*SC + bz));  a = sigmoid(kh*SC + bh)
                        nc.scalar.activation(
                            ct[:], kz[:], AF.Sigmoid, bias=nbz_sb[:, bc],
                            scale=-SC,
                        )
                        nc.scalar.activation(
                            at[:], kh[:], AF.Sigmoid, bias=bh_sb[:, bc],
                            scale=SC,
                        )
                        # m = kh*SC + bh + 0.5
                        nc.scalar.activation(
                            mt[:], kh[:], AF.Identity, bias=bhh_sb[:, bc],
                            scale=SC,
                        )
                    tl = inter.tile([P, sb], F16, tag="tl")
                    zt = inter.tile([P, sb], F16, tag="z")
                    if "dve" not in ablate:
                        nc.vector.tensor_max(tl[:], at[:], mt[:])
                        # z = 1 - c  (tensor_scalar, 4x fp16 mode)
                        nc.vector.tensor_scalar(
                            zt[:], ct[:], -1.0, 1.0, OP.mult, OP.add
                        )
                    vt = inter.tile([P, sb], F16, tag="v")
                    if "pool" not in ablate:
                        eng = nc.gpsimd if v_engine == "pool" else nc.vector
                        eng.tensor_tensor(vt[:], zt[:], tl[:], OP.mult)

                    ot = outp.tile([P, sb], F16, tag=f"o{hi}")
                    if "scan" not in ablate:
                        init = (
                            g0[:, bc] if sbi == 0
                            else prev_out[hi][:, sb - 1:sb]
                        )
                        nc.vector.tensor_tensor_scan(
                            ot[:], ct[:], vt[:], init, op0=OP.mult, op1=OP.add
                        )
                        prev_out[hi] = ot
                    if "outdma" not in ablate:
                        nc.sync.dma_start(
                            out=hT_v[:, hi, sbi * sb:(sbi + 1) * sb], in_=ot[:]
                        )
    nc.finalize()
    return nc


def _get_program():
    if "nc" not in _CACHE:
        _CACHE["nc"] = _build_program()
    return _CACHE["nc"]


def _q8(v):
    return v.astype(mybir.dt.np(F8))


def prep_core_inputs(x_b, h0_b, consts, mm_style=None):
    """Per-core input map for batch row x_b (S, D), h0_b (H,)."""
    if mm_style is None:
        mm_style = MM_STYLE
    xT = np.ascontiguousarray(x_b.T).astype(np.float32)      # (D, S)
    if mm_style == "f32r":
        xt8 = xT
    else:
        xh = _q8(xT)
        xl = _q8(xT - xh.astype(np.float32))
        # (d, (nsb two s)): per s-block, the hi block then the lo block
        xt8 = np.stack(
            [xh.reshape(D, NSB, SB), xl.reshape(D, NSB, SB)], axis=2
        ).reshape(D, 2 * S)
    return {
        "xt8": xt8,
        "h0g": np.ascontiguousarray(h0_b.reshape(HT, P).T),
        **consts,
    }


def prep_const_inputs(Wz, bz, Wh, bh, mm_style=None):
    """Weight/bias tensors shared by all cores."""
    if mm_style is None:
        mm_style = MM_STYLE
    consts = {}
    for name, W in (("z", Wz), ("h", Wh)):
        WT = np.ascontiguousarray(W.T).astype(np.float32)  # (D, H)
        if mm_style == "f32r":
            consts[f"w{name}8"] = WT
            continue
        W32T = WT * 32.0
        Whi = _q8(W32T)
        Wlo = _q8(W32T - Whi.astype(np.float32))
        if mm_style == "kd":
            consts[f"w{name}8"] = Whi
        else:
            consts[f"w{name}8"] = np.stack([Whi, Whi], axis=1).reshape(D, 2 * H)
        consts[f"w{name}l8"] = Wlo
    consts["bzg"] = np.ascontiguousarray(bz.reshape(HT, P).T)
    consts["bhg"] = np.ascontiguousarray(bh.reshape(HT, P).T)
    return consts


def run(inputs, **kw):
    """Run on hardware; returns (output (B,S,H) fp32, BassKernelResults)."""
    x = np.asarray(inputs["x"], dtype=np.float32)
    h0 = np.asarray(inputs["h0"], dtype=np.float32)
    consts = prep_const_inputs(
        np.asarray(inputs["Wz"], dtype=np.float32),
        np.asarray(inputs["bz"], dtype=np.float32),
        np.asarray(inputs["Wh"], dtype=np.float32),
        np.asarray(inputs["bh"], dtype=np.float32),
    )
    in_maps = [
        prep_core_inputs(x[b], h0[b, 0], consts) for b in range(N_CORES)
    ]

    nc = _get_program()
    res = run_bass_kernel_spmd(nc, in_maps, core_ids=list(range(N_CORES)), **kw)
    out = np.stack(
        [res.results[b]["ht"].astype(np.float32).T for b in range(N_CORES)],
        axis=0,
    )
    return np.ascontiguousarray(out), res


def kernel(**inputs):
    out, _ = run(inputs)
    return out
